# revision 37
# baseline (speedup 1.0000x reference)
"""Trainium2 Bass kernel for nn_ModelNew_3556232922055 (dense_cnn).

Semantics (per image):
  y8[j]    = conv2d_valid(x, weight[:8]) + bias[:8]          (8,126,126)
  acc[co]  = max over (ci,kh,kw) of 2*W[co,ci,kh,kw]*y8[ci,h+kh,w+kw]
             (out-of-range taps excluded at the bottom/right borders)
  out      = min over co of acc                              (1,126,126)

Sharding: data-parallel over batch, 1 image per NeuronCore (8 cores).

Device mapping per core (v3):
  - host-built im2col X72 [72, 16128] bf16, DMA'd in 4 double-buffered
    pieces so the conv starts early
  - conv as a k=72 bf16 matmul pass -> PSUM -> ACT evac (+bias) into
    Y8REP [128, 16128] bf16 where partition p = ci*16 + r holds y8[ci]
  - step 2 in 3 row-bands:
      tap 0: DVE tensor_scalar (4x bf16) initializes pacc[half]
      taps 1-5: product on ACT (scalar.mul, 1x) -> DVE tensor_tensor max
      taps 6-8: product on DVE tensor_scalar (4x, full-width contiguous)
                -> DVE tensor_tensor max (2x)
    (GpSimd is ~9G elem/s on trn2: never use it. scalar_tensor_tensor
    has no 2x uop: fused product+max runs 1x = slower than TS+TT.)
  - reduce per band in 7-row groups: PE-transpose 128x128 chunks of both
    pacc halves into one PSUM tile -> single ACT copy to SBUF -> DVE
    free-dim folds (max over ci, min over halves + co) -> OUT[w,h]
  - final PE transpose -> DMA out (126,126) f32.
"""

import numpy as np
from contextlib import ExitStack

import concourse.bass as bass
import concourse.tile as tile
from concourse import bacc, mybir
from concourse import masks
from concourse.bass_utils import run_bass_kernel_spmd

F32 = mybir.dt.float32
BF16 = mybir.dt.bfloat16

DT_Y = BF16    # y8 replicas
DT_ACC = BF16  # pacc accumulators / products

H = W = 128
CIN = 8
COUT = 32
K = 3
OH = OW = 126
NPIX = H * OH          # 16128 flat pixels (h*128+w), h<126
YPAD = 4               # y8 tail pad so full-width shifted reads stay in-bounds
NCORES = 8
CHUNK = 504            # conv free-dim chunk (<=512, 32*504=16128)
NPIECE = 8             # x72 DMA pieces (each 2016 px = 4 chunks)
GROUP = 7              # reduce-phase chunks (=output rows) per psum tile
NPAIR = 3              # psum tiles folded together per DVE fold pass
BANDS = [(0, 42), (42, 84), (84, 126)]
BH_MAX = 42

TAPS = [(kh, kw) for kh in range(K) for kw in range(K)]
# Per-(tap, half) product engine: "A" = ACT (scalar.mul, 1x), "V" = DVE
# TS 4x.  9 A-units / 7 V-units balances ACT ~190us vs DVE ~195us.
A_UNITS = [(1, 0), (1, 1), (2, 0), (2, 1), (3, 0), (3, 1), (4, 0), (4, 1),
           (5, 0)]
V_UNITS = [(6, 0), (6, 1), (7, 0), (7, 1), (8, 0), (8, 1), (5, 1)]
# DVE consumption order: V-units interleave with A-TTs so the DVE never
# outruns ACT's 1x product rate (A-products consumed in issue order).
SCHED = ["V", "V", "A", "V", "A", "V", "A", "V", "A", "V", "A", "V",
         "A", "A", "A", "A"]


def _r3(t, h0, nh, w0, nw):
    """3D region view [128, nh, nw] of a [128, NPIX] tile at rows h0, cols w0."""
    return t[:, 0:NPIX].rearrange("p (h w) -> p h w", w=W)[
        :, h0 : h0 + nh, w0 : w0 + nw
    ]


def build_program():
    nc = bacc.Bacc()

    # x72: host-built im2col, x72[(kh*3+kw)*8+ci, pix] = x[ci, pix+kh*128+kw]
    x_d = nc.declare_dram_parameter("x72", [72, NPIX], BF16, isOutput=False)
    # consts: [:, 0:18] scal, [:, 18:19] bias128
    c_d = nc.declare_dram_parameter("consts", [128, 19], F32, isOutput=False)
    w_d = nc.declare_dram_parameter("w1b", [72, 128], BF16, isOutput=False)
    out_d = nc.declare_dram_parameter("out", [OH, OW], F32, isOutput=True)

    with ExitStack() as ctx:
        tc = ctx.enter_context(tile.TileContext(nc))

        consts = ctx.enter_context(tc.tile_pool(name="consts", bufs=1))
        big = ctx.enter_context(tc.tile_pool(name="big", bufs=1))

        constst = consts.tile([128, 19], F32)
        nc.sync.dma_start(constst[:], c_d[:])
        scalt = constst[:, 0:18]
        biast = constst[:, 18:19]
        w1t = consts.tile([72, 128], BF16)
        nc.sync.dma_start(w1t[:], w_d[:])
        ident = consts.tile([128, 128], DT_ACC)
        masks.make_identity(nc, ident[:])
        ident_f32 = consts.tile([128, 128], F32)
        masks.make_identity(nc, ident_f32[:])

        y8 = big.tile([128, NPIX + YPAD], DT_Y)
        paccs = []
        for half in range(2):
            pacc = big.tile([128, NPIX], DT_ACC, tag=f"pacc{half}")
            paccs.append(pacc)
        outt = big.tile([128, OH], F32)  # OUT[w, h]

        # Separate product pools: ACT-produced (apool) and DVE-produced
        # (vpool).  Separate so a vpool WAR never waits on an A-tap TT that
        # sits later in the DVE stream (shared rotation would deadlock).
        apool = ctx.enter_context(tc.tile_pool(name="apool", bufs=3))
        vpool = ctx.enter_context(tc.tile_pool(name="vpool", bufs=2))
        redpool = ctx.enter_context(tc.tile_pool(name="redpool", bufs=2))
        xp = ctx.enter_context(tc.tile_pool(name="xp", bufs=3))
        psum = ctx.enter_context(tc.tile_pool(name="psum", bufs=2, space="PSUM"))
        psred = ctx.enter_context(tc.tile_pool(name="psred", bufs=2, space="PSUM"))

        # --- load + conv: y8rep[p = ci*16+r] = y8[ci], bf16 ---
        # x72 streamed in 4 double-buffered pieces so conv starts early.
        PIECE = NPIX // NPIECE
        for h in range(NPIECE):
            xh = xp.tile([72, PIECE], BF16, tag="xh")
            nc.sync.dma_start(out=xh[:], in_=x_d[:, h * PIECE : (h + 1) * PIECE])
            for c in range(PIECE // CHUNK):
                n0 = h * PIECE + c * CHUNK
                ps = psum.tile([128, CHUNK], F32, tag="convps")
                nc.tensor.matmul(
                    ps[:], lhsT=w1t[:], rhs=xh[:, c * CHUNK : (c + 1) * CHUNK],
                    start=True, stop=True,
                )
                nc.scalar.activation(
                    y8[:, n0 : n0 + CHUNK], ps[:],
                    mybir.ActivationFunctionType.Identity,
                    bias=biast, scale=1.0,
                )

        mu = mybir.AluOpType.mult
        mx = mybir.AluOpType.max
        mn = mybir.AluOpType.min

        def reduce_pair(c0, rows):
            """Fold chunks [c0, c0+rows) of both pacc halves into outt.

            rows = NPAIR*GROUP output rows; each GROUP-row block is PE-
            transposed into its own 2-bank PSUM tile, then ACT-copied into
            an s-major SBUF tile pt[p][s=2][g][c][ci][co] (one strided copy
            per group).  s-major means every DVE fold runs ONCE across all
            g blocks: 3 ci folds, 1 half-min, 4 co folds = 8 ops total.
            """
            ngrp = rows // GROUP
            GB = GROUP * 128  # cols per (s, g) block
            pt = redpool.tile([128, NPAIR * 2 * GB], DT_ACC, tag="PT")
            ptv = pt[:].rearrange("p (s g x) -> p s g x", s=2, g=NPAIR)
            for g in range(ngrp):
                g0 = c0 + g * GROUP
                psT = psred.tile([128, 2 * GB], DT_ACC, tag="psT")
                for half in range(2):
                    for j in range(GROUP):
                        nc.tensor.transpose(
                            psT[:, (half * GROUP + j) * 128
                                : (half * GROUP + j + 1) * 128],
                            paccs[half][:, (g0 + j) * 128 : (g0 + j + 1) * 128],
                            ident[:],
                        )
                nc.scalar.copy(
                    ptv[:, :, g, :],
                    psT[:].rearrange("p (s x) -> p s x", s=2),
                )
            # merged ci folds across all (s, g, c) rows at once
            q = 2 * ngrp * GROUP
            v = pt[:, 0 : q * 128].rearrange(
                "p (q ci co) -> p q ci co", q=q, ci=8)
            nc.vector.tensor_tensor(
                v[:, :, 0:4, :], v[:, :, 0:4, :], v[:, :, 4:8, :], mx)
            nc.vector.tensor_tensor(
                v[:, :, 0:2, :], v[:, :, 0:2, :], v[:, :, 2:4, :], mx)
            nc.vector.tensor_tensor(
                v[:, :, 0:1, :], v[:, :, 0:1, :], v[:, :, 1:2, :], mx)
            # min(A-half, B-half) in one op: s is the outermost dim
            gc2 = ngrp * GROUP
            vs = pt[:, 0 : q * 128].rearrange(
                "p (s c ci co) -> p s c ci co", s=2, c=gc2, ci=8)
            nc.vector.tensor_tensor(
                vs[:, 0:1, :, 0:1, :], vs[:, 0:1, :, 0:1, :],
                vs[:, 1:2, :, 0:1, :], mn)
            w2 = vs[:, 0, :, 0, :]  # [p, c = ngrp*GROUP rows, co16]
            nc.vector.tensor_tensor(w2[:, :, 0:8], w2[:, :, 0:8], w2[:, :, 8:16], mn)
            nc.vector.tensor_tensor(w2[:, :, 0:4], w2[:, :, 0:4], w2[:, :, 4:8], mn)
            nc.vector.tensor_tensor(w2[:, :, 0:2], w2[:, :, 0:2], w2[:, :, 2:4], mn)
            nc.vector.tensor_tensor(
                outt[:, c0 : c0 + rows], w2[:, :, 0:1], w2[:, :, 1:2], mn)

        # --- step 2, banded; reduce band k while band k+1 computes ---
        # DVE stream order per band: tap0 -> V-taps (self-paced TS 4x + TT)
        # -> A-tap TTs.  ACT builds A-products concurrently (issued first in
        # its stream); max is commutative so any tap order is valid, and the
        # V-stretch hides ACT's product latency.
        def piece_dims(t, p0, p1):
            """Valid rows/cols of tap t within band piece [p0, p1)."""
            kh, kw = TAPS[t]
            nh = min(p1, OH - kh) - p0
            nw = OW - kw
            src_full = y8[:, (p0 + kh) * W + kw : (p0 + kh + nh) * W + kw]
            return nh, nw, src_full

        for h0, h1 in BANDS:
            bh = h1 - h0
            # tap 0, half 1 runs on ACT (rebalance: ACT ~187 vs DVE ~194);
            # first in the ACT band stream since it initializes pacc1.
            nc.scalar.mul(
                paccs[1][:, h0 * W : h1 * W], y8[:, h0 * W : h1 * W],
                scalt[:, 9:10],
            )
            # A-products on ACT, full-band (issued first so ACT starts
            # immediately; apool WAR rotation throttles its run-ahead)
            aprods = {}
            for t, half in A_UNITS:
                nh, nw, src_full = piece_dims(t, h0, h1)
                p = apool.tile([128, BH_MAX * W], DT_ACC, tag="P")
                sc = scalt[:, half * 9 + t : half * 9 + t + 1]
                nc.scalar.mul(p[:, 0 : nh * W], src_full, sc)
                aprods[(t, half)] = (
                    p[:].rearrange("p (h w) -> p h w", w=W)[:, 0:nh, 0:nw]
                )
            # tap 0 half 0 covers the full band (incl. junk cols): 4x TS
            nc.vector.tensor_scalar(
                _r3(paccs[0], h0, bh, 0, W), _r3(y8, h0, bh, 0, W),
                scalt[:, 0:1], None, mu,
            )
            # interleaved max-accumulation per SCHED
            vq = list(V_UNITS)
            aq = list(A_UNITS)
            for kind in SCHED:
                if kind == "V":
                    t, half = vq.pop(0)
                    nh, nw, src_full = piece_dims(t, h0, h1)
                    p = vpool.tile([128, BH_MAX * W], DT_ACC, tag="V")
                    sc = scalt[:, half * 9 + t : half * 9 + t + 1]
                    nc.vector.tensor_scalar(
                        p[:, 0 : nh * W], src_full, sc, None, mu)
                    p3 = p[:].rearrange("p (h w) -> p h w", w=W)[:, 0:nh, 0:nw]
                    acc3 = _r3(paccs[half], h0, nh, 0, nw)
                    nc.vector.tensor_tensor(acc3, acc3, p3, mx)
                else:
                    t, half = aq.pop(0)
                    nh, nw, _ = piece_dims(t, h0, h1)
                    acc3 = _r3(paccs[half], h0, nh, 0, nw)
                    nc.vector.tensor_tensor(acc3, acc3, aprods[(t, half)], mx)
            for c0 in range(h0, h1, NPAIR * GROUP):
                reduce_pair(c0, min(NPAIR * GROUP, h1 - c0))

        # transpose OUT[w,h] -> [h,w] and write out
        pso = psred.tile([128, 128], F32, tag="pso")
        nc.tensor.transpose(pso[0:OH, :], outt[:, 0:OH], ident_f32[:])
        res = consts.tile([128, 128], F32)
        nc.scalar.copy(res[0:OH, :], pso[0:OH, :])
        nc.sync.dma_start(out_d[:, :], res[0:OH, 0:OW])

    nc.compile()
    return nc


def host_tiles(weight, bias):
    weight = np.asarray(weight, np.float32)
    bias = np.asarray(bias, np.float32)
    w1rep = np.zeros((72, 128), np.float32)
    for kh in range(K):
        for kw in range(K):
            for ci_in in range(CIN):
                t = (kh * K + kw) * CIN + ci_in
                for ci_out in range(CIN):
                    w1rep[t, ci_out * 16 : ci_out * 16 + 16] = weight[
                        ci_out, ci_in, kh, kw
                    ]
    bias128 = np.repeat(bias[:CIN], 16).astype(np.float32).reshape(128, 1)
    scal = np.zeros((128, 18), np.float32)
    for p in range(128):
        ci = p // 16
        co_lo = p % 16
        for half in range(2):
            co = co_lo + 16 * half
            for t, (kh, kw) in enumerate(TAPS):
                scal[p, half * 9 + t] = 2.0 * weight[co, ci, kh, kw]
    consts = np.zeros((128, 19), np.float32)
    consts[:, 0:18] = scal
    consts[:, 18:19] = bias128
    return consts, w1rep


def im2col_host(xb):
    """xb: (8,128,128) f32 -> (72, NPIX) bf16 with junk tail cols zeroed."""
    x72 = np.zeros((72, NPIX), np.float32)
    L = NPIX - 2
    flat = xb.reshape(-1)
    for kh in range(K):
        for kw in range(K):
            for ci in range(CIN):
                t = (kh * K + kw) * CIN + ci
                off = kh * W + kw
                x72[t, :L] = flat[ci * H * W + off : ci * H * W + off + L]
    return x72


def _to_bf16(a):
    import ml_dtypes
    return a.astype(ml_dtypes.bfloat16)


_CACHE = {}


def _get_program():
    if "nc" not in _CACHE:
        _CACHE["nc"] = build_program()
    return _CACHE["nc"]


def run_spmd(x, weight, bias, **kw):
    x = np.ascontiguousarray(np.asarray(x, np.float32))
    consts, w1rep = host_tiles(weight, bias)
    w1b = _to_bf16(w1rep)
    nc = _get_program()
    in_maps = [
        {"x72": _to_bf16(im2col_host(x[b])), "consts": consts, "w1b": w1b}
        for b in range(NCORES)
    ]
    res = run_bass_kernel_spmd(nc, in_maps, list(range(NCORES)), **kw)
    out = np.stack([res.results[b]["out"] for b in range(NCORES)])
    return out[:, None, :, :].astype(np.float32), res


def kernel(x, weight, bias):
    out, _ = run_spmd(x, weight, bias)
    return out


if __name__ == "__main__":
    rng = np.random.default_rng(0)
    x = rng.standard_normal((8, CIN, H, W), dtype=np.float32)
    wt = rng.uniform(-0.1, 0.1, (COUT, CIN, K, K)).astype(np.float32)
    bs = rng.uniform(-0.1, 0.1, COUT).astype(np.float32)
    print(kernel(x, wt, bs).shape)


# revision 40
# speedup vs baseline: 1.0267x; 1.0267x over previous
"""Trainium2 Bass kernel for nn_ModelNew_3556232922055 (dense_cnn).

Semantics (per image):
  y8[j]    = conv2d_valid(x, weight[:8]) + bias[:8]          (8,126,126)
  acc[co]  = max over (ci,kh,kw) of 2*W[co,ci,kh,kw]*y8[ci,h+kh,w+kw]
             (out-of-range taps excluded at the bottom/right borders)
  out      = min over co of acc                              (1,126,126)

Sharding: data-parallel over batch, 1 image per NeuronCore (8 cores).

Device mapping per core (v3):
  - host-built im2col X72 [72, 16128] bf16, DMA'd in 4 double-buffered
    pieces so the conv starts early
  - conv as a k=72 bf16 matmul pass -> PSUM -> ACT evac (+bias) into
    Y8REP [128, 16128] bf16 where partition p = ci*16 + r holds y8[ci]
  - step 2 in 3 row-bands:
      tap 0: DVE tensor_scalar (4x bf16) initializes pacc[half]
      taps 1-5: product on ACT (scalar.mul, 1x) -> DVE tensor_tensor max
      taps 6-8: product on DVE tensor_scalar (4x, full-width contiguous)
                -> DVE tensor_tensor max (2x)
    (GpSimd is ~9G elem/s on trn2: never use it. scalar_tensor_tensor
    has no 2x uop: fused product+max runs 1x = slower than TS+TT.)
  - reduce per band in 7-row groups: PE-transpose 128x128 chunks of both
    pacc halves into one PSUM tile -> single ACT copy to SBUF -> DVE
    free-dim folds (max over ci, min over halves + co) -> OUT[w,h]
  - final PE transpose -> DMA out (126,126) f32.
"""

import numpy as np
from contextlib import ExitStack

import concourse.bass as bass
import concourse.tile as tile
from concourse import bacc, mybir
from concourse import masks
from concourse.bass_utils import run_bass_kernel_spmd

F32 = mybir.dt.float32
BF16 = mybir.dt.bfloat16

DT_Y = BF16    # y8 replicas
DT_ACC = BF16  # pacc accumulators / products

H = W = 128
CIN = 8
COUT = 32
K = 3
OH = OW = 126
NPIX = H * OH          # 16128 flat pixels (h*128+w), h<126
YPAD = 4               # y8 tail pad so full-width shifted reads stay in-bounds
NCORES = 8
CHUNK = 504            # conv free-dim chunk (<=512, 32*504=16128)
NPIECE = 8             # x72 DMA pieces (each 2016 px = 4 chunks)
GROUP = 7              # reduce-phase chunks (=output rows) per psum tile
NPAIR = 3              # psum tiles folded together per DVE fold pass
BANDS = [(0, 42), (42, 84), (84, 126)]
BH_MAX = 42

TAPS = [(kh, kw) for kh in range(K) for kw in range(K)]
# Per-(tap, half) product engine: "A" = ACT (scalar.mul, 1x), "V" = DVE
# TS 4x.  9 A-units / 7 V-units balances ACT ~190us vs DVE ~195us.
A_UNITS = [(1, 0), (1, 1), (2, 0), (2, 1), (3, 0), (3, 1), (4, 0), (4, 1),
           (5, 0), (8, 1)]
V_UNITS = [(6, 0), (6, 1), (7, 0), (7, 1), (8, 0), (5, 1)]
# DVE consumption order: V-units interleave with A-TTs so the DVE never
# outruns ACT's 1x product rate (A-products consumed in issue order).
SCHED = ["V", "V", "A", "V", "A", "V", "A", "V", "A", "V", "A", "A",
         "A", "A", "A", "A"]


def _r3(t, h0, nh, w0, nw):
    """3D region view [128, nh, nw] of a [128, NPIX] tile at rows h0, cols w0."""
    return t[:, 0:NPIX].rearrange("p (h w) -> p h w", w=W)[
        :, h0 : h0 + nh, w0 : w0 + nw
    ]


def build_program():
    nc = bacc.Bacc()

    # x72: host-built im2col, x72[(kh*3+kw)*8+ci, pix] = x[ci, pix+kh*128+kw]
    x_d = nc.declare_dram_parameter("x72", [72, NPIX], BF16, isOutput=False)
    # consts: [:, 0:18] scal, [:, 18:19] bias128
    c_d = nc.declare_dram_parameter("consts", [128, 19], F32, isOutput=False)
    w_d = nc.declare_dram_parameter("w1b", [72, 128], BF16, isOutput=False)
    out_d = nc.declare_dram_parameter("out", [OH, OW], F32, isOutput=True)

    with ExitStack() as ctx:
        tc = ctx.enter_context(tile.TileContext(nc))

        consts = ctx.enter_context(tc.tile_pool(name="consts", bufs=1))
        big = ctx.enter_context(tc.tile_pool(name="big", bufs=1))

        constst = consts.tile([128, 19], F32)
        nc.sync.dma_start(constst[:], c_d[:])
        scalt = constst[:, 0:18]
        biast = constst[:, 18:19]
        w1t = consts.tile([72, 128], BF16)
        nc.sync.dma_start(w1t[:], w_d[:])
        ident = consts.tile([128, 128], DT_ACC)
        masks.make_identity(nc, ident[:])
        ident_f32 = consts.tile([128, 128], F32)
        masks.make_identity(nc, ident_f32[:])

        y8 = big.tile([128, NPIX + YPAD], DT_Y)
        paccs = []
        for half in range(2):
            pacc = big.tile([128, NPIX], DT_ACC, tag=f"pacc{half}")
            paccs.append(pacc)
        outt = big.tile([128, OH], F32)  # OUT[w, h]

        # Separate product pools: ACT-produced (apool) and DVE-produced
        # (vpool).  Separate so a vpool WAR never waits on an A-tap TT that
        # sits later in the DVE stream (shared rotation would deadlock).
        apool = ctx.enter_context(tc.tile_pool(name="apool", bufs=3))
        vpool = ctx.enter_context(tc.tile_pool(name="vpool", bufs=2))
        redpool = ctx.enter_context(tc.tile_pool(name="redpool", bufs=2))
        xp = ctx.enter_context(tc.tile_pool(name="xp", bufs=3))
        psum = ctx.enter_context(tc.tile_pool(name="psum", bufs=2, space="PSUM"))
        psred = ctx.enter_context(tc.tile_pool(name="psred", bufs=2, space="PSUM"))

        # --- load + conv: y8rep[p = ci*16+r] = y8[ci], bf16 ---
        # x72 streamed in 4 double-buffered pieces so conv starts early.
        PIECE = NPIX // NPIECE
        for h in range(NPIECE):
            xh = xp.tile([72, PIECE], BF16, tag="xh")
            nc.sync.dma_start(out=xh[:], in_=x_d[:, h * PIECE : (h + 1) * PIECE])
            for c in range(PIECE // CHUNK):
                n0 = h * PIECE + c * CHUNK
                ps = psum.tile([128, CHUNK], F32, tag="convps")
                nc.tensor.matmul(
                    ps[:], lhsT=w1t[:], rhs=xh[:, c * CHUNK : (c + 1) * CHUNK],
                    start=True, stop=True,
                )
                nc.scalar.activation(
                    y8[:, n0 : n0 + CHUNK], ps[:],
                    mybir.ActivationFunctionType.Identity,
                    bias=biast, scale=1.0,
                )

        mu = mybir.AluOpType.mult
        mx = mybir.AluOpType.max
        mn = mybir.AluOpType.min

        def reduce_pair(c0, rows):
            """Fold chunks [c0, c0+rows) of both pacc halves into outt.

            rows = NPAIR*GROUP output rows; each GROUP-row block is PE-
            transposed into its own 2-bank PSUM tile, then ACT-copied into
            an s-major SBUF tile pt[p][s=2][g][c][ci][co] (one strided copy
            per group).  s-major means every DVE fold runs ONCE across all
            g blocks: 3 ci folds, 1 half-min, 4 co folds = 8 ops total.
            """
            ngrp = rows // GROUP
            GB = GROUP * 128  # cols per (s, g) block
            pt = redpool.tile([128, NPAIR * 2 * GB], DT_ACC, tag="PT")
            ptv = pt[:].rearrange("p (s g x) -> p s g x", s=2, g=NPAIR)
            for g in range(ngrp):
                g0 = c0 + g * GROUP
                psT = psred.tile([128, 2 * GB], DT_ACC, tag="psT")
                for half in range(2):
                    for j in range(GROUP):
                        nc.tensor.transpose(
                            psT[:, (half * GROUP + j) * 128
                                : (half * GROUP + j + 1) * 128],
                            paccs[half][:, (g0 + j) * 128 : (g0 + j + 1) * 128],
                            ident[:],
                        )
                nc.scalar.copy(
                    ptv[:, :, g, :],
                    psT[:].rearrange("p (s x) -> p s x", s=2),
                )
            # merged ci folds across all (s, g, c) rows at once
            q = 2 * ngrp * GROUP
            v = pt[:, 0 : q * 128].rearrange(
                "p (q ci co) -> p q ci co", q=q, ci=8)
            nc.vector.tensor_tensor(
                v[:, :, 0:4, :], v[:, :, 0:4, :], v[:, :, 4:8, :], mx)
            nc.vector.tensor_tensor(
                v[:, :, 0:2, :], v[:, :, 0:2, :], v[:, :, 2:4, :], mx)
            nc.vector.tensor_tensor(
                v[:, :, 0:1, :], v[:, :, 0:1, :], v[:, :, 1:2, :], mx)
            # min(A-half, B-half) in one op: s is the outermost dim
            gc2 = ngrp * GROUP
            vs = pt[:, 0 : q * 128].rearrange(
                "p (s c ci co) -> p s c ci co", s=2, c=gc2, ci=8)
            nc.vector.tensor_tensor(
                vs[:, 0:1, :, 0:1, :], vs[:, 0:1, :, 0:1, :],
                vs[:, 1:2, :, 0:1, :], mn)
            w2 = vs[:, 0, :, 0, :]  # [p, c = ngrp*GROUP rows, co16]
            nc.vector.tensor_tensor(w2[:, :, 0:8], w2[:, :, 0:8], w2[:, :, 8:16], mn)
            nc.vector.tensor_tensor(w2[:, :, 0:4], w2[:, :, 0:4], w2[:, :, 4:8], mn)
            nc.vector.tensor_tensor(w2[:, :, 0:2], w2[:, :, 0:2], w2[:, :, 2:4], mn)
            nc.vector.tensor_tensor(
                outt[:, c0 : c0 + rows], w2[:, :, 0:1], w2[:, :, 1:2], mn)

        # --- step 2, banded; reduce band k while band k+1 computes ---
        # DVE stream order per band: tap0 -> V-taps (self-paced TS 4x + TT)
        # -> A-tap TTs.  ACT builds A-products concurrently (issued first in
        # its stream); max is commutative so any tap order is valid, and the
        # V-stretch hides ACT's product latency.
        def piece_dims(t, p0, p1):
            """Valid rows/cols of tap t within band piece [p0, p1)."""
            kh, kw = TAPS[t]
            nh = min(p1, OH - kh) - p0
            nw = OW - kw
            src_full = y8[:, (p0 + kh) * W + kw : (p0 + kh + nh) * W + kw]
            return nh, nw, src_full

        for h0, h1 in BANDS:
            bh = h1 - h0
            # A-products on ACT, full-band (issued first so ACT starts
            # immediately; apool WAR rotation throttles its run-ahead)
            aprods = {}
            for t, half in A_UNITS:
                nh, nw, src_full = piece_dims(t, h0, h1)
                p = apool.tile([128, BH_MAX * W], DT_ACC, tag="P")
                sc = scalt[:, half * 9 + t : half * 9 + t + 1]
                nc.scalar.mul(p[:, 0 : nh * W], src_full, sc)
                aprods[(t, half)] = (
                    p[:].rearrange("p (h w) -> p h w", w=W)[:, 0:nh, 0:nw]
                )
            # tap 0 covers the full band (incl. junk cols 126/127): 4x TS
            for half in range(2):
                nc.vector.tensor_scalar(
                    _r3(paccs[half], h0, bh, 0, W), _r3(y8, h0, bh, 0, W),
                    scalt[:, half * 9 : half * 9 + 1], None, mu,
                )
            # interleaved max-accumulation per SCHED
            vq = list(V_UNITS)
            aq = list(A_UNITS)
            for kind in SCHED:
                if kind == "V":
                    t, half = vq.pop(0)
                    nh, nw, src_full = piece_dims(t, h0, h1)
                    p = vpool.tile([128, BH_MAX * W], DT_ACC, tag="V")
                    sc = scalt[:, half * 9 + t : half * 9 + t + 1]
                    nc.vector.tensor_scalar(
                        p[:, 0 : nh * W], src_full, sc, None, mu)
                    p3 = p[:].rearrange("p (h w) -> p h w", w=W)[:, 0:nh, 0:nw]
                    acc3 = _r3(paccs[half], h0, nh, 0, nw)
                    nc.vector.tensor_tensor(acc3, acc3, p3, mx)
                else:
                    t, half = aq.pop(0)
                    nh, nw, _ = piece_dims(t, h0, h1)
                    acc3 = _r3(paccs[half], h0, nh, 0, nw)
                    nc.vector.tensor_tensor(acc3, acc3, aprods[(t, half)], mx)
            for c0 in range(h0, h1, NPAIR * GROUP):
                reduce_pair(c0, min(NPAIR * GROUP, h1 - c0))

        # transpose OUT[w,h] -> [h,w] and write out
        pso = psred.tile([128, 128], F32, tag="pso")
        nc.tensor.transpose(pso[0:OH, :], outt[:, 0:OH], ident_f32[:])
        res = consts.tile([128, 128], F32)
        nc.scalar.copy(res[0:OH, :], pso[0:OH, :])
        nc.sync.dma_start(out_d[:, :], res[0:OH, 0:OW])

    nc.compile()
    return nc


def host_tiles(weight, bias):
    weight = np.asarray(weight, np.float32)
    bias = np.asarray(bias, np.float32)
    w1rep = np.zeros((72, 128), np.float32)
    for kh in range(K):
        for kw in range(K):
            for ci_in in range(CIN):
                t = (kh * K + kw) * CIN + ci_in
                for ci_out in range(CIN):
                    w1rep[t, ci_out * 16 : ci_out * 16 + 16] = weight[
                        ci_out, ci_in, kh, kw
                    ]
    bias128 = np.repeat(bias[:CIN], 16).astype(np.float32).reshape(128, 1)
    scal = np.zeros((128, 18), np.float32)
    for p in range(128):
        ci = p // 16
        co_lo = p % 16
        for half in range(2):
            co = co_lo + 16 * half
            for t, (kh, kw) in enumerate(TAPS):
                scal[p, half * 9 + t] = 2.0 * weight[co, ci, kh, kw]
    consts = np.zeros((128, 19), np.float32)
    consts[:, 0:18] = scal
    consts[:, 18:19] = bias128
    return consts, w1rep


def im2col_host(xb):
    """xb: (8,128,128) f32 -> (72, NPIX) bf16 with junk tail cols zeroed."""
    x72 = np.zeros((72, NPIX), np.float32)
    L = NPIX - 2
    flat = xb.reshape(-1)
    for kh in range(K):
        for kw in range(K):
            for ci in range(CIN):
                t = (kh * K + kw) * CIN + ci
                off = kh * W + kw
                x72[t, :L] = flat[ci * H * W + off : ci * H * W + off + L]
    return x72


def _to_bf16(a):
    import ml_dtypes
    return a.astype(ml_dtypes.bfloat16)


_CACHE = {}


def _get_program():
    if "nc" not in _CACHE:
        _CACHE["nc"] = build_program()
    return _CACHE["nc"]


def run_spmd(x, weight, bias, **kw):
    x = np.ascontiguousarray(np.asarray(x, np.float32))
    consts, w1rep = host_tiles(weight, bias)
    w1b = _to_bf16(w1rep)
    nc = _get_program()
    in_maps = [
        {"x72": _to_bf16(im2col_host(x[b])), "consts": consts, "w1b": w1b}
        for b in range(NCORES)
    ]
    res = run_bass_kernel_spmd(nc, in_maps, list(range(NCORES)), **kw)
    out = np.stack([res.results[b]["out"] for b in range(NCORES)])
    return out[:, None, :, :].astype(np.float32), res


def kernel(x, weight, bias):
    out, _ = run_spmd(x, weight, bias)
    return out


if __name__ == "__main__":
    rng = np.random.default_rng(0)
    x = rng.standard_normal((8, CIN, H, W), dtype=np.float32)
    wt = rng.uniform(-0.1, 0.1, (COUT, CIN, K, K)).astype(np.float32)
    bs = rng.uniform(-0.1, 0.1, COUT).astype(np.float32)
    print(kernel(x, wt, bs).shape)
